# revision 1
# baseline (speedup 1.0000x reference)
"""Trainium2 Bass kernel for nn_ContrastiveLoss (binary-label supervised
contrastive loss over an 8192x8192 cosine-similarity matrix).

Math: with binary targets, each sample has class q = 2*tt + tp in {0..3}.
pos_mask(i,j) <=> class(j) == q_i^1, neg_mask(i,j) <=> class(j) == q_i^2.
Rows of classes {0,3} only ever need columns of classes {1,2} and vice
versa, so half the similarity matrix is never needed.  Per row i:
    loss_i = valid_i * ( sum_{j pos} sim_ij/(T*pos_cnt) - log(Epos+Eneg) )
where Epos+Eneg = sum over j in both needed classes of exp(sim_ij/T), and
sum_{j pos} sim_ij = y_i . S_pos with S_pos the sum of normalized features
of the pos class (computed on device).

Sharding (data-parallel over anchors): cores 0-3 take {0,3}-class rows,
cores 4-7 take {1,2}-class rows; each core gets the two j-class segments it
needs (zero-padded to a fixed width).  Device computes everything O(B^2);
host does only O(B) index bookkeeping and the final 8-way partial-sum.
"""

import sys

if "/opt/trn_rl_repo" not in sys.path:  # harmless if concourse already importable
    sys.path.insert(0, "/opt/trn_rl_repo")

from contextlib import ExitStack

import numpy as np

import concourse.bass as bass
import concourse.bacc as bacc
import concourse.tile as tile
from concourse import masks, mybir
from concourse.bass_utils import run_bass_kernel_spmd

F32 = mybir.dt.float32
BF16 = mybir.dt.bfloat16
AX = mybir.AxisListType
AF = mybir.ActivationFunctionType
ALU = mybir.AluOpType

B, D = 8192, 128
TEMP = 0.1
N_CORES = 8
F_CHUNKS = 9               # 9 f-chunks of 128 rows per core (capacity 1152)
FP = F_CHUNKS * 128
GROUP = 1536               # dots/exp group width (3 PSUM banks, double-buffered)

_program_cache = {}

_COMBINED_SET = "natural_log_exp_and_others"


def _patch_act_tables():
    """Make Bacc's table-load pass pick the set holding BOTH Ln and Exp.
    The default greedy pick loads exp_and_others / natural_log alternately
    (~1.3us per switch, 13 switches in the unpatched kernel)."""
    import concourse.bacc as _bacc
    if getattr(_bacc, "_act_tables_patched", False):
        return
    real = _bacc.get_activation_tables

    def patched(arch):
        tabs = real(arch)
        if _COMBINED_SET in tabs:
            keep = tabs[_COMBINED_SET]
            for name, fns in tabs.items():
                if name != _COMBINED_SET and (fns & keep):
                    tabs[name] = fns - keep
        return tabs

    _bacc.get_activation_tables = patched
    _bacc._act_tables_patched = True


def build_program(NJ: int, W1: int):
    """One SPMD program; all 8 cores run it on their own inputs."""
    _patch_act_tables()
    nc = bacc.Bacc("TRN2", target_bir_lowering=False, debug=False,
                   num_devices=N_CORES)
    JC = NJ // 128

    ffeat = nc.declare_dram_parameter("ffeat", [FP, D], F32, isOutput=False)
    jfeat = nc.declare_dram_parameter("jfeat", [NJ, D], F32, isOutput=False)
    wls_in = nc.declare_dram_parameter("wls", [128, F_CHUNKS, 2], F32, isOutput=False)
    vmask_in = nc.declare_dram_parameter("vmask", [128, F_CHUNKS], F32, isOutput=False)
    lbias_in = nc.declare_dram_parameter("lbias", [128, 1], F32, isOutput=False)
    partial = nc.declare_dram_parameter("partial", [1, 1], F32, isOutput=True)

    NC_TOT = F_CHUNKS + JC  # all chunks: f first, then j

    # dots groups covering [0, NJ)
    groups = []
    off = 0
    while off < NJ:
        w = min(GROUP, NJ - off)
        groups.append((off, w))
        off += w
    NG = len(groups)

    with ExitStack() as ctx:
        tc = ctx.enter_context(tile.TileContext(nc))
        consts = ctx.enter_context(tc.tile_pool(name="consts", bufs=1))
        sqpool = ctx.enter_context(tc.tile_pool(name="sqpool", bufs=3))
        ypool = ctx.enter_context(tc.tile_pool(name="ychunk", bufs=3))
        persist = ctx.enter_context(tc.tile_pool(name="persist", bufs=1))
        scratch = ctx.enter_context(tc.tile_pool(name="scratch", bufs=2))
        dots_ps = ctx.enter_context(tc.tile_pool(name="dots", bufs=2, space="PSUM"))
        tp_ps = ctx.enter_context(tc.tile_pool(name="tp", bufs=1, space="PSUM"))
        s_ps = ctx.enter_context(tc.tile_pool(name="sp", bufs=1, space="PSUM"))

        # ---- constants ----
        ident = consts.tile([128, 128], BF16)
        masks.make_identity(nc, ident)
        ones_col = consts.tile([128, 1], F32)
        nc.vector.memset(ones_col, 1.0)
        eps_col = consts.tile([128, 1], F32)
        nc.vector.memset(eps_col, 1e-20)

        # ---- small inputs ----
        wls_t = persist.tile([128, F_CHUNKS, 2], F32)
        nc.sync.dma_start(out=wls_t, in_=wls_in[:])
        vmask_t = persist.tile([128, F_CHUNKS], F32)
        nc.sync.dma_start(out=vmask_t, in_=vmask_in[:])
        lbias_t = persist.tile([128, 1], F32)
        nc.sync.dma_start(out=lbias_t, in_=lbias_in[:])

        # ---- persistent state ----
        YTf = persist.tile([128, FP], BF16)        # normalized f-features, [d, i]
        YTj = persist.tile([128, NJ], BF16)        # normalized j-features, [d, j]
        nsq = persist.tile([128, NC_TOT], F32)
        lnn = persist.tile([128, NC_TOT], F32)
        rinv = persist.tile([128, NC_TOT], F32)
        Aslots = persist.tile([128, F_CHUNKS, NG], F32)
        LSall = persist.tile([128, F_CHUNKS, 2], F32)
        S_sb = persist.tile([128, 2], BF16)

        # all raw feature chunks live in one persistent buffer, loaded by a
        # handful of large DMAs (walrus allows only one sync-wait per DMA, so
        # slot-reuse WAR waits on small per-chunk DMAs are not an option)
        x_all = persist.tile([128, NC_TOT, D], F32)

        # ---- prep: per 128-row chunk: load, nsq, rsqrt, normalize,
        #      transpose (and for j-chunks, accumulate S) ----
        def chunk_meta(t):
            if t < F_CHUNKS:
                return YTf[:, t * 128 : (t + 1) * 128], False
            c = t - F_CHUNKS
            return YTj[:, c * 128 : (c + 1) * 128], True

        RSQ_GRP = 8
        for g0 in range(0, NC_TOT, RSQ_GRP):
            g1 = min(g0 + RSQ_GRP, NC_TOT)
            # group load: at most two DMAs (f- and j-source parts)
            if g0 < F_CHUNKS:
                f1 = min(g1, F_CHUNKS)
                nc.sync.dma_start(
                    out=x_all[:, g0:f1, :],
                    in_=ffeat[:].rearrange("(c p) d -> p c d", p=128)[:, g0:f1, :],
                )
                if g1 > F_CHUNKS:
                    nc.sync.dma_start(
                        out=x_all[:, F_CHUNKS:g1, :],
                        in_=jfeat[:].rearrange("(c p) d -> p c d", p=128)[
                            :, 0 : g1 - F_CHUNKS, :],
                    )
            else:
                nc.sync.dma_start(
                    out=x_all[:, g0:g1, :],
                    in_=jfeat[:].rearrange("(c p) d -> p c d", p=128)[
                        :, g0 - F_CHUNKS : g1 - F_CHUNKS, :],
                )
            gw = g1 - g0
            sq = sqpool.tile([128, RSQ_GRP, D], F32, tag="sq")
            nc.vector.tensor_mul(sq[:, :gw, :], x_all[:, g0:g1, :],
                                 x_all[:, g0:g1, :])
            nc.vector.reduce_sum(out=nsq[:, g0:g1], in_=sq[:, :gw, :],
                                 axis=AX.X, op=ALU.add)
            # rinv = exp(-0.5 * ln(nsq + eps)); Ln/Exp share one ACT table set
            nc.scalar.activation(out=lnn[:, g0:g1], in_=nsq[:, g0:g1],
                                 func=AF.Ln, bias=eps_col)
            nc.scalar.activation(out=rinv[:, g0:g1], in_=lnn[:, g0:g1],
                                 func=AF.Exp, scale=-0.5)
            for t in range(g0, g1):
                yt_dst, _ = chunk_meta(t)
                y = ypool.tile([128, D], BF16, tag="y")
                nc.vector.tensor_scalar_mul(y, x_all[:, t, :], rinv[:, t : t + 1])
                tp = tp_ps.tile([128, 128], BF16, tag="tp")
                nc.tensor.transpose(tp, y, ident)
                nc.vector.tensor_copy(out=yt_dst, in_=tp)
        # S[d, s] = sum of normalized features in segment s: plain free-dim
        # reductions over the transposed j-features (zero pads contribute 0)
        S_f32 = persist.tile([128, 2], F32)
        nc.vector.reduce_sum(out=S_f32[:, 0:1], in_=YTj[:, 0:W1],
                             axis=AX.X, op=ALU.add)
        nc.vector.reduce_sum(out=S_f32[:, 1:2], in_=YTj[:, W1:NJ],
                             axis=AX.X, op=ALU.add)
        nc.vector.tensor_copy(out=S_sb, in_=S_f32)

        # ---- LS[i, s] = y_i . S_s  (sum of sim over segment s) ----
        for c in range(F_CHUNKS):
            ls_ps = s_ps.tile([128, 2], F32, tag="sp")
            nc.tensor.matmul(ls_ps, lhsT=YTf[:, c * 128 : (c + 1) * 128],
                             rhs=S_sb, start=True, stop=True)
            nc.vector.tensor_copy(out=LSall[:, c, :], in_=ls_ps)

        # ---- main loop: dots + fused exp/accumulate ----
        for c in range(F_CHUNKS):
            lhsT = YTf[:, c * 128 : (c + 1) * 128]
            for gi, (j0, gw) in enumerate(groups):
                dp = dots_ps.tile([128, GROUP], F32, tag="dots")
                b0 = 0
                while b0 < gw:
                    bw = min(512, gw - b0)
                    nc.tensor.matmul(
                        dp[:, b0 : b0 + bw], lhsT=lhsT,
                        rhs=YTj[:, j0 + b0 : j0 + b0 + bw],
                        start=True, stop=True,
                    )
                    b0 += bw
                es = scratch.tile([128, GROUP], BF16, tag="es")
                nc.scalar.activation(
                    out=es[:, :gw], in_=dp[:, :gw], func=AF.Exp,
                    scale=1.0 / TEMP, accum_out=Aslots[:, c, gi : gi + 1],
                )

        # ---- finalization ----
        Dsum = persist.tile([128, F_CHUNKS], F32)
        nc.vector.reduce_sum(out=Dsum, in_=Aslots, axis=AX.X, op=ALU.add)
        ln_all = persist.tile([128, F_CHUNKS], F32)
        nc.scalar.activation(out=ln_all, in_=Dsum, func=AF.Ln, bias=lbias_t)
        wtmp = persist.tile([128, F_CHUNKS, 2], F32)
        nc.vector.tensor_mul(wtmp, LSall, wls_t)
        LSsel = persist.tile([128, F_CHUNKS], F32)
        nc.vector.reduce_sum(out=LSsel, in_=wtmp, axis=AX.X, op=ALU.add)
        vtmp = persist.tile([128, F_CHUNKS], F32)
        nc.vector.tensor_mul(vtmp, ln_all, vmask_t)
        contrib = persist.tile([128, F_CHUNKS], F32)
        nc.vector.tensor_sub(contrib, LSsel, vtmp)
        ctot = persist.tile([128, 1], F32)
        nc.vector.reduce_sum(out=ctot, in_=contrib, axis=AX.X, op=ALU.add)
        gr = s_ps.tile([1, 1], F32, tag="sp")
        nc.tensor.matmul(gr, lhsT=ones_col, rhs=ctot, start=True, stop=True)
        out_sb = consts.tile([1, 1], F32)
        nc.scalar.copy(out=out_sb, in_=gr)
        nc.sync.dma_start(out=partial[:], in_=out_sb)

    nc.compile()
    return nc


def host_shard(features, data_ix, targets_t, targets_p):
    tt = np.asarray(targets_t)[np.asarray(data_ix)].astype(np.int32)
    tp = np.asarray(targets_p)[np.asarray(data_ix)].astype(np.int32)
    q = 2 * tt + tp
    cnt = np.bincount(q, minlength=4)
    pos_cnt = cnt[q ^ 1]
    neg_cnt = cnt[q ^ 2]
    valid = (pos_cnt > 0) & (neg_cnt > 0)

    idx = [np.nonzero(q == c)[0] for c in range(4)]
    a_rows = np.concatenate([idx[0], idx[3]])      # cores 0-3
    b_rows = np.concatenate([idx[1], idx[2]])      # cores 4-7
    assert len(a_rows) <= 4 * FP and len(b_rows) <= 4 * FP

    W1 = (max(len(idx[1]), len(idx[0])) + 127) // 128 * 128
    W2 = (max(len(idx[2]), len(idx[3])) + 127) // 128 * 128
    NJ = W1 + W2
    feats = np.asarray(features, np.float32)

    def seg(c, W):
        out = np.zeros((W, D), np.float32)
        out[: len(idx[c])] = feats[idx[c]]
        return out

    jfeat_sides = [
        np.concatenate([seg(1, W1), seg(2, W2)]),  # for {0,3} rows
        np.concatenate([seg(0, W1), seg(3, W2)]),  # for {1,2} rows
    ]
    npad = [NJ - cnt[1] - cnt[2], NJ - cnt[0] - cnt[3]]

    in_maps = []
    for k in range(N_CORES):
        side = 0 if k < 4 else 1
        rows = (a_rows if side == 0 else b_rows)[k % 4 * FP : (k % 4 + 1) * FP]
        n = len(rows)
        ffeat = np.zeros((FP, D), np.float32)
        ffeat[:n] = feats[rows]
        wls = np.zeros((FP, 2), np.float32)
        vmask = np.zeros(FP, np.float32)
        seg_classes = (1, 2) if side == 0 else (0, 3)
        pos_class = q[rows] ^ 1
        vmask[:n] = valid[rows]
        for s, c in enumerate(seg_classes):
            m = (pos_class == c) & valid[rows]
            wls[:n][m, s] = 1.0 / (TEMP * pos_cnt[rows][m])
        in_maps.append({
            "ffeat": ffeat,
            "jfeat": jfeat_sides[side],
            "wls": np.ascontiguousarray(
                wls.reshape(F_CHUNKS, 128, 2).transpose(1, 0, 2)),
            "vmask": np.ascontiguousarray(
                vmask.reshape(F_CHUNKS, 128).transpose(1, 0)),
            "lbias": np.full((128, 1), -float(npad[side]), np.float32),
        })
    return in_maps, NJ, W1


def run_on_device(in_maps, NJ, W1, **kw):
    key = (NJ, W1)
    if key not in _program_cache:
        _program_cache[key] = build_program(NJ, W1)
    nc = _program_cache[key]
    return run_bass_kernel_spmd(nc, in_maps, list(range(N_CORES)), **kw)


def kernel(features, data_ix, targets_t, targets_p):
    in_maps, NJ, W1 = host_shard(features, data_ix, targets_t, targets_p)
    res = run_on_device(in_maps, NJ, W1)
    total = sum(float(r["partial"][0, 0]) for r in res.results)
    return np.float32(-total / B)


if __name__ == "__main__":
    import importlib.util

    spec = importlib.util.spec_from_file_location(
        "reference", "/root/problem/reference.py")
    ref = importlib.util.module_from_spec(spec)
    spec.loader.exec_module(ref)
    inputs = {k: np.asarray(v) for k, v in ref.setup_inputs().items()}
    out = kernel(**inputs)
    print("kernel loss:", out)



# revision 2
# speedup vs baseline: 1.6415x; 1.6415x over previous
"""Trainium2 Bass kernel for nn_ContrastiveLoss (binary-label supervised
contrastive loss over an 8192x8192 cosine-similarity matrix).

Math: with binary targets, each sample has class q = 2*tt + tp in {0..3}.
pos_mask(i,j) <=> class(j) == q_i^1, neg_mask(i,j) <=> class(j) == q_i^2, so
rows of classes {0,3} only need columns of classes {1,2} and vice versa.
Per row: loss_i = valid_i * (y_i.S_pos/(T*pos_cnt) - log(sum_j exp(sim_ij/T)))
where the j-sum runs over the two needed classes.

Device does the O(B^2) part: row-block x col-block dots (PE), exp + row-sum
split across the Scalar engine (native Exp+accum) and the Vector engine (two
custom DVE ops: deg-4 poly p~=exp(t/16), then p^16 with fused accumulate).
Host does O(B*D) prep (normalize, transpose, bf16 cast, class packing) and
the O(B) finalization (log, pos-term, masking).

Sharding: cores 0-3 take {0,3}-class anchor rows, cores 4-7 take {1,2}; each
core computes 9 chunks x NJe cols of exp-sums and returns 54 partial-sum
columns; host combines.
"""

import sys

if "/opt/trn_rl_repo" not in sys.path:
    sys.path.insert(0, "/opt/trn_rl_repo")

from contextlib import ExitStack
from operator import add

import numpy as np
import ml_dtypes

import concourse.bass as bass
import concourse.bacc as bacc
import concourse.tile as tile
from concourse import mybir
from concourse.bass_utils import run_bass_kernel_spmd
from concourse.dve_ops import (DveOp, OPS, CUSTOM_DVE_SPECS,
                               _SUB_OPCODE_FOR_NAME, _CUSTOM_DVE_ROW_BASE)
from concourse.dve_spec import (Spec, Src0, Src1, C0, C1, C2, C3, One, sq,
                                lower as dve_lower, _has_src1,
                                _spill_c3_to_src1)
from concourse.dve_uop import DveOpSpec

F32 = mybir.dt.float32
BF16 = mybir.dt.bfloat16
AF = mybir.ActivationFunctionType

B, D = 8192, 128
TEMP = 0.1
N_CORES = 8
F_CHUNKS = 9                # 9 f-chunks of 128 rows per core (capacity 1152)
FP = F_CHUNKS * 128
G0, G1 = 1536, 1536         # first two dots groups (3 PSUM banks each)

# deg-4 fit of e^u on [-0.625, 0.625] (a0=1), in x=sim: p=1+x(b1+x(b2+x(b3+x b4)))
PB = (0.6247442364692688, 0.1953597515821457,
      0.041675373911857605, 0.006334216333925724)

_program_cache = {}


def _register(name, spec):
    if name in _SUB_OPCODE_FOR_NAME:
        return next(op for op in OPS if op.name == name)
    row = _CUSTOM_DVE_ROW_BASE + len(OPS)
    assert row < 0x20
    _SUB_OPCODE_FOR_NAME[name] = row
    shas = {}
    for ver in ("v3", "v4"):
        uops = dve_lower(spec, ver=ver)
        shas[ver] = DveOpSpec(name=name, opcode=row, uops=uops,
                              rd1_en=_has_src1(spec)).sha(ver)
    op = DveOp(name, spec, subdim=False, uops_sha=shas)
    OPS.append(op)
    CUSTOM_DVE_SPECS[name] = spec
    return op


def _exp_ops():
    body = One + Src0 * (C0 + Src0 * (C1 + Src0 * (C2 + Src0 * C3)))
    p_ref = lambda in0, in1, s0, s1, imm2: (
        1 + in0 * (s0 + in0 * (s1 + in0 * (imm2 + in0 * in1)))).astype(np.float32)
    op_poly = _register("EXPQ16_POLY_ANT",
                        Spec(body=_spill_c3_to_src1(body), reference=p_ref))

    def pow16_ref(in0, in1, s0, s1, imm2):
        b = (in0.astype(np.float32) ** 16).astype(np.float32)
        return b, b.reshape(b.shape[0], -1).sum(-1, keepdims=True)

    op_pow = _register("POW16_ACC_ANT",
                       Spec(body=sq(sq(sq(sq(Src0)))), accum=add,
                            reference=pow16_ref))
    return op_poly, op_pow


def _act_split(gw):
    """Columns the ACT engine takes from a gw-wide group (rest -> DVE, 2cpe)."""
    ca = (2.083 * gw + 246 - 422) / 2.916
    return max(0, min(gw, int(ca) & ~1))


def build_program(NJe: int):
    op_poly, op_pow = _exp_ops()
    nc = bacc.Bacc("TRN2", target_bir_lowering=False, debug=False,
                   num_devices=N_CORES)

    ytf = nc.declare_dram_parameter("ytf", [128, FP], BF16, isOutput=False)
    ytj = nc.declare_dram_parameter("ytj", [128, NJe], BF16, isOutput=False)
    acc_out = nc.declare_dram_parameter("acc", [128, 6 * F_CHUNKS], F32,
                                        isOutput=True)

    groups = [(0, G0), (G0, G1), (G0 + G1, NJe - G0 - G1)]
    assert 512 < groups[2][1] <= 1536

    with ExitStack() as ctx:
        tc = ctx.enter_context(tile.TileContext(nc))
        persist = ctx.enter_context(tc.tile_pool(name="persist", bufs=1))
        pqpool = ctx.enter_context(tc.tile_pool(name="pq", bufs=2))
        dots_ps = ctx.enter_context(tc.tile_pool(name="dots", bufs=2,
                                                 space="PSUM"))

        YTf = persist.tile([128, FP], BF16)
        YTj = persist.tile([128, NJe], BF16)
        nc.sync.dma_start(out=YTf, in_=ytf[:])
        nc.sync.dma_start(out=YTj[:, 0:G0], in_=ytj[:, 0:G0])
        nc.sync.dma_start(out=YTj[:, G0:G0 + G1], in_=ytj[:, G0:G0 + G1])
        nc.sync.dma_start(out=YTj[:, G0 + G1:NJe], in_=ytj[:, G0 + G1:NJe])

        A = persist.tile([128, F_CHUNKS, 3, 2], F32)
        c3t = persist.tile([128, 1], F32)
        nc.vector.memset(c3t, float(PB[3]))
        es = persist.tile([128, 1152], BF16)    # ACT exp dump (discarded)
        pd = persist.tile([128, 704], BF16)     # op2 dump (discarded)

        for gi, (j0, gw) in enumerate(groups):
            ca = _act_split(gw)
            for c in range(F_CHUNKS):
                lhsT = YTf[:, c * 128:(c + 1) * 128]
                dp = dots_ps.tile([128, 1536], F32, tag="dots")
                b0 = 0
                while b0 < gw:
                    bw = min(512, gw - b0)
                    nc.tensor.matmul(dp[:, b0:b0 + bw], lhsT=lhsT,
                                     rhs=YTj[:, j0 + b0:j0 + b0 + bw],
                                     start=True, stop=True)
                    b0 += bw
                nc.scalar.activation(out=es[:, :ca], in_=dp[:, :ca],
                                     func=AF.Exp, scale=1.0 / TEMP,
                                     accum_out=A[:, c, gi, 0:1])
                pq = pqpool.tile([128, 704], BF16, tag="pq")
                nc.vector._custom_dve(op_poly, out=pq[:, :gw - ca],
                                      in0=dp[:, ca:gw], in1=c3t,
                                      s0=float(PB[0]), s1=float(PB[1]),
                                      imm2=float(PB[2]))
                nc.vector._custom_dve(op_pow, out=pd[:, :gw - ca],
                                      in0=pq[:, :gw - ca],
                                      accum_out=A[:, c, gi, 1:2])

        nc.sync.dma_start(out=acc_out[:], in_=A)

    nc.compile()
    return nc


def host_shard(features, data_ix, targets_t, targets_p):
    tt = np.asarray(targets_t)[np.asarray(data_ix)].astype(np.int32)
    tp = np.asarray(targets_p)[np.asarray(data_ix)].astype(np.int32)
    q = 2 * tt + tp
    cnt = np.bincount(q, minlength=4)
    pos_cnt = cnt[q ^ 1]
    neg_cnt = cnt[q ^ 2]
    valid = (pos_cnt > 0) & (neg_cnt > 0)

    feats = np.asarray(features, np.float32)
    norms = np.sqrt((feats * feats).sum(1))
    y = feats / np.maximum(norms, 1e-8)[:, None]
    ybf = y.astype(ml_dtypes.bfloat16)

    idx = [np.nonzero(q == c)[0] for c in range(4)]
    a_rows = np.concatenate([idx[0], idx[3]])      # cores 0-3
    b_rows = np.concatenate([idx[1], idx[2]])      # cores 4-7
    assert len(a_rows) <= 4 * FP and len(b_rows) <= 4 * FP

    W1 = (max(len(idx[1]), len(idx[0])) + 1) & ~1
    W2 = (max(len(idx[2]), len(idx[3])) + 1) & ~1
    NJe = W1 + W2
    if NJe - G0 - G1 <= 512:          # keep last group in (512, 1536]
        NJe = G0 + G1 + 514
    npad = [NJe - cnt[1] - cnt[2], NJe - cnt[0] - cnt[3]]

    def jside(c1, c2):
        out = np.zeros((128, NJe), ml_dtypes.bfloat16)
        out[:, :len(idx[c1])] = ybf[idx[c1]].T
        out[:, W1:W1 + len(idx[c2])] = ybf[idx[c2]].T
        return out

    ytj_sides = [jside(1, 2), jside(0, 3)]

    in_maps, core_rows = [], []
    for k in range(N_CORES):
        side = 0 if k < 4 else 1
        rows = (a_rows if side == 0 else b_rows)[k % 4 * FP:(k % 4 + 1) * FP]
        ytf = np.zeros((128, FP), ml_dtypes.bfloat16)
        ytf[:, :len(rows)] = ybf[rows].T
        in_maps.append({"ytf": ytf, "ytj": ytj_sides[side]})
        core_rows.append(rows)
    meta = dict(q=q, pos_cnt=pos_cnt, valid=valid, y=y, idx=idx, npad=npad,
                core_rows=core_rows)
    return in_maps, NJe, meta


def finalize(results, meta):
    q, pos_cnt, valid, y = meta["q"], meta["pos_cnt"], meta["valid"], meta["y"]
    denom = np.zeros(B, np.float64)
    for k, r in enumerate(results):
        rows = meta["core_rows"][k]
        a = np.asarray(r["acc"], np.float64).reshape(128, F_CHUNKS, 6)
        per_row = a.sum(2).T.reshape(-1)            # [FP] chunk-major rows
        side = 0 if k < 4 else 1
        denom[rows] = per_row[:len(rows)] - meta["npad"][side]
    S = np.stack([y[meta["idx"][c]].sum(0) for c in range(4)])   # [4, D]
    LS = (y @ S.T)[np.arange(B), q ^ 1]
    log_denom = np.log(np.maximum(denom, 1e-300))
    mlp = np.where(valid, LS / (TEMP * np.maximum(pos_cnt, 1)) - log_denom, 0.0)
    return np.float32(-mlp.sum() / B)


def run_on_device(in_maps, NJe, **kw):
    if NJe not in _program_cache:
        _program_cache[NJe] = build_program(NJe)
    return run_bass_kernel_spmd(_program_cache[NJe], in_maps,
                                list(range(N_CORES)), **kw)


def kernel(features, data_ix, targets_t, targets_p):
    in_maps, NJe, meta = host_shard(features, data_ix, targets_t, targets_p)
    res = run_on_device(in_maps, NJe)
    return finalize(res.results, meta)


if __name__ == "__main__":
    import importlib.util

    spec = importlib.util.spec_from_file_location(
        "reference", "/root/problem/reference.py")
    ref = importlib.util.module_from_spec(spec)
    spec.loader.exec_module(ref)
    inputs = {k: np.asarray(v) for k, v in ref.setup_inputs().items()}
    out = kernel(**inputs)
    print("kernel loss:", out)
